# revision 4
# baseline (speedup 1.0000x reference)
"""GCN layer (project -> degree-norm -> gather/scatter-sum -> norm -> relu) on 8 trn2 cores.

Strategy (dst-sharded, per the sharding hint, but with gather-side replication
instead of an all-to-all):
  - 16 dst-shards of 3125 nodes; core c owns shards 2c and 2c+1.
  - Per shard, host renumbers the referenced src nodes compactly (~31.6k distinct,
    +1 zero token) so gather indices fit dma_gather's int16 contract.
  - Device per shard: project h_compact @ W (bf16, PE) -> *norm -> bf16 node table
    in DRAM [NTOK, 128]; SWDGE dma_gather pulls per-edge rows (256B) into SBUF
    in (dst-tile, round) slot order; PE accumulates each dst tile's rounds in
    PSUM via an identity-stationary matmul; ScalarE fuses *norm_dst + ReLU.
  - Edges land in a dense (tile, round, slot) grid: slot = rank of dst within its
    128-node tile (nodes sorted by in-degree so round counts are even), round =
    per-dst edge occurrence. Holes point at token 0 == zero row, so the PSUM sum
    is exact with no scatter collisions anywhere.
All float math happens on device; the host only shards/permutes/casts.
"""

import math

import numpy as np
import ml_dtypes

import concourse.bacc as bacc
import concourse.bass as bass
import concourse.mybir as mybir
import concourse.tile as tile
from concourse.bass_utils import run_bass_kernel_spmd

N_NODES = 50000
D_IN = 256
D_OUT = 96
N_CORES = 8
SHARDS_PER_CORE = 2
P = 128
MAX_GROUPS_PER_BATCH = 64

BF16 = mybir.dt.bfloat16
F32 = mybir.dt.float32
I16 = mybir.dt.int16


def _plan(h, norm, weight, src, dst, n_cores=N_CORES, shards_per_core=SHARDS_PER_CORE):
    """Host-side index/layout preprocessing. Returns per-core input maps plus the
    metadata needed to build the (single, SPMD-shared) bass program and to
    reassemble the full output."""
    n_shards = n_cores * shards_per_core
    n_nodes = h.shape[0]
    shard_n = int(math.ceil(n_nodes / n_shards))
    n_dtiles = int(math.ceil(shard_n / P))
    shard_pad = n_dtiles * P

    src = np.asarray(src).astype(np.int64).ravel()
    dst = np.asarray(dst).astype(np.int64).ravel()
    norm_f = np.asarray(norm, dtype=np.float32).reshape(n_nodes)
    hT = np.ascontiguousarray(np.asarray(h, dtype=np.float32).T)  # [D_IN, n_nodes]

    shard_of = dst // shard_n

    shards = []
    max_ntok = 0
    R_shard = np.zeros((n_shards, n_dtiles), dtype=np.int64)
    for s in range(n_shards):
        lo = s * shard_n
        sel = shard_of == s
        es = src[sel]
        ed = dst[sel] - lo
        n_local = min(shard_n, n_nodes - lo)
        deg = np.bincount(ed, minlength=n_local)
        perm = np.argsort(-deg, kind="stable")  # degree desc
        rank = np.empty(n_local, dtype=np.int64)
        rank[perm] = np.arange(n_local)
        er = rank[ed]
        eorder = np.argsort(er, kind="stable")
        es2 = es[eorder]
        er2 = er[eorder]
        # occurrence index of each edge within its dst's run
        if len(er2):
            starts = np.r_[0, np.flatnonzero(np.diff(er2)) + 1]
            counts = np.diff(np.r_[starts, len(er2)])
            occ = np.arange(len(er2)) - np.repeat(starts, counts)
        else:
            occ = np.zeros(0, dtype=np.int64)
        tj = er2 // P
        np.maximum.at(R_shard[s], tj, occ + 1)
        uniq = np.unique(es2)
        ntok = len(uniq) + 1
        assert ntok <= 32767, f"shard {s}: {ntok} tokens exceeds int16 gather range"
        max_ntok = max(max_ntok, ntok)
        tok = np.searchsorted(uniq, es2) + 1
        shards.append(dict(perm=perm, uniq=uniq, tj=tj, slot=er2 % P, occ=occ,
                           tok=tok, n_local=n_local))

    R_j = np.maximum(R_shard.max(axis=0), 1)  # uniform rounds grid across shards
    goff = np.concatenate([[0], np.cumsum(R_j)])
    G = int(goff[-1])
    NTOK = int(math.ceil(max_ntok / P) * P)
    T_PROJ = NTOK // P

    # batches of whole dst tiles, each <= MAX_GROUPS_PER_BATCH groups
    batches = []  # (g0, [(j, R_j, local_off)...], ngroups)
    cur, cum = [], 0
    for j in range(n_dtiles):
        r = int(R_j[j])
        if cur and cum + r > MAX_GROUPS_PER_BATCH:
            batches.append((int(goff[cur[0]]), [(jj, int(R_j[jj]), int(goff[jj] - goff[cur[0]])) for jj in cur], cum))
            cur, cum = [], 0
        cur.append(j)
        cum += r
    if cur:
        batches.append((int(goff[cur[0]]), [(jj, int(R_j[jj]), int(goff[jj] - goff[cur[0]])) for jj in cur], cum))

    # per-shard device arrays
    for s, sd in enumerate(shards):
        uniq = sd["uniq"]
        ntok = len(uniq) + 1
        hc = np.zeros((D_IN, NTOK), dtype=np.float32)
        hc[:, 1:ntok] = hT[:, uniq]
        ht_tiled = np.ascontiguousarray(
            hc.reshape(2, P, T_PROJ, P).transpose(2, 1, 0, 3)
        ).astype(ml_dtypes.bfloat16)  # [T_PROJ, 128(k), 2(j), 128(tok)]

        nsc = np.zeros(NTOK, dtype=np.float32)
        nsc[1:ntok] = norm_f[uniq]
        nsrc = np.ascontiguousarray(nsc.reshape(T_PROJ, P).T)  # [128, T_PROJ]

        nd = np.zeros(shard_pad, dtype=np.float32)
        nd[: sd["n_local"]] = norm_f[s * shard_n + sd["perm"]]
        ndst = np.ascontiguousarray(nd.reshape(n_dtiles, P).T)  # [128, n_dtiles]

        idx_flat = np.zeros(G * P, dtype=np.int64)
        pos = (goff[sd["tj"]] + sd["occ"]) * P + sd["slot"]
        idx_flat[pos] = sd["tok"]
        w16 = idx_flat.reshape(G * 8, 16).T.astype(np.int16)  # [16, G*8]
        gidx = np.ascontiguousarray(np.tile(w16, (8, 1)))  # [128, G*8] replicated

        sd["ht"] = ht_tiled
        sd["nsrc"] = nsrc
        sd["ndst"] = ndst
        sd["gidx"] = gidx

    w2 = np.zeros((D_IN, P), dtype=np.float32)
    w2[:, :D_OUT] = np.asarray(weight, dtype=np.float32)
    # [128(k), 2(j), 128(col)] so partition dim (k within chunk) comes first
    wpad = np.ascontiguousarray(
        w2.reshape(2, P, P).transpose(1, 0, 2)
    ).astype(ml_dtypes.bfloat16)
    ident = np.eye(P, dtype=np.float32).astype(ml_dtypes.bfloat16)

    in_maps = []
    for c in range(n_cores):
        m = {"w": wpad, "ident": ident}
        for y in range(shards_per_core):
            sd = shards[c * shards_per_core + y]
            m[f"ht{y}"] = sd["ht"]
            m[f"nsrc{y}"] = sd["nsrc"]
            m[f"ndst{y}"] = sd["ndst"]
            m[f"gidx{y}"] = sd["gidx"]
        in_maps.append(m)

    meta = dict(
        n_cores=n_cores, shards_per_core=shards_per_core, n_shards=n_shards,
        shard_n=shard_n, n_dtiles=n_dtiles, shard_pad=shard_pad,
        NTOK=NTOK, T_PROJ=T_PROJ, G=G, batches=batches,
    )
    return in_maps, meta, shards


def _build_nc(meta):
    spc = meta["shards_per_core"]
    T_PROJ = meta["T_PROJ"]
    NTOK = meta["NTOK"]
    G = meta["G"]
    n_dtiles = meta["n_dtiles"]
    shard_pad = meta["shard_pad"]
    batches = meta["batches"]
    max_b_groups = max(b[2] for b in batches)

    nc = bacc.Bacc("TRN2", target_bir_lowering=False, debug=False)

    w_d = nc.dram_tensor("w", [P, 2, P], BF16, kind="ExternalInput")
    id_d = nc.dram_tensor("ident", [P, P], BF16, kind="ExternalInput")
    ht_d, nsrc_d, ndst_d, gidx_d = [], [], [], []
    for y in range(spc):
        ht_d.append(nc.dram_tensor(f"ht{y}", [T_PROJ, P, 2, P], BF16, kind="ExternalInput"))
        nsrc_d.append(nc.dram_tensor(f"nsrc{y}", [P, T_PROJ], F32, kind="ExternalInput"))
        ndst_d.append(nc.dram_tensor(f"ndst{y}", [P, n_dtiles], F32, kind="ExternalInput"))
        gidx_d.append(nc.dram_tensor(f"gidx{y}", [P, G * 8], I16, kind="ExternalInput"))
    out_d = nc.dram_tensor("out", [spc * shard_pad, D_OUT], F32, kind="ExternalOutput")

    with tile.TileContext(nc) as tc:
        with (
            tc.tile_pool(name="const", bufs=1) as cpool,
            tc.tile_pool(name="ht", bufs=4) as htpool,
            tc.tile_pool(name="stage", bufs=4) as stpool,
            tc.tile_pool(name="msgs", bufs=3) as mpool,
            tc.tile_pool(name="outsb", bufs=4) as opool,
            tc.tile_pool(name="ppsum", bufs=3, space="PSUM") as pppool,
            tc.tile_pool(name="apsum", bufs=3, space="PSUM") as appool,
            tc.tile_pool(name="dram", bufs=1, space="DRAM") as dpool,
        ):
            w_sb = cpool.tile([P, 2, P], BF16, tag="w")
            nc.sync.dma_start(w_sb[:], w_d[:])
            id_sb = cpool.tile([P, P], BF16, tag="ident")
            nc.sync.dma_start(id_sb[:], id_d[:])
            nsrc_sb, ndst_sb, gidx_sb = [], [], []
            for y in range(spc):
                t1 = cpool.tile([P, T_PROJ], F32, tag=f"nsrc{y}")
                nc.sync.dma_start(t1[:], nsrc_d[y][:])
                nsrc_sb.append(t1)
                t2 = cpool.tile([P, n_dtiles], F32, tag=f"ndst{y}")
                nc.sync.dma_start(t2[:], ndst_d[y][:])
                ndst_sb.append(t2)
                t3 = cpool.tile([P, G * 8], I16, tag=f"gidx{y}")
                nc.sync.dma_start(t3[:], gidx_d[y][:])
                gidx_sb.append(t3)

            for y in range(spc):
                table = dpool.tile([NTOK, P], BF16, tag=f"table{y}")
                # projection: table[tok] = bf16((h_compact @ W) * norm_src)
                for t in range(T_PROJ):
                    ht_t = htpool.tile([P, 2, P], BF16, tag="ht")
                    nc.sync.dma_start(ht_t[:], ht_d[y][t])
                    pp = pppool.tile([P, P], F32, tag="pp")
                    for j in range(2):
                        nc.tensor.matmul(
                            out=pp[:], lhsT=ht_t[:, j, :], rhs=w_sb[:, j, :],
                            start=(j == 0), stop=(j == 1),
                        )
                    stg = stpool.tile([P, P], BF16, tag="stage")
                    nc.vector.tensor_scalar(
                        out=stg[:], in0=pp[:], scalar1=nsrc_sb[y][:, t : t + 1],
                        scalar2=None, op0=mybir.AluOpType.mult,
                    )
                    nc.sync.dma_start(table[t * P : (t + 1) * P, :], stg[:])

                # gather + per-dst-tile segment sum + norm + relu + store
                for g0, tl, ng in batches:
                    msgs = mpool.tile([P, max_b_groups, P], BF16, tag="msgs")
                    n_idx = ng * P
                    nc.gpsimd.dma_gather(
                        msgs[:, :ng, :], table[:], gidx_sb[y][:, g0 * 8 : (g0 + ng) * 8],
                        n_idx, n_idx, P, single_packet=False,
                    )
                    for j, rj, loff in tl:
                        ap = appool.tile([P, D_OUT], F32, tag="ap")
                        for r in range(rj):
                            nc.tensor.matmul(
                                out=ap[:], lhsT=id_sb[:], rhs=msgs[:, loff + r, 0:D_OUT],
                                start=(r == 0), stop=(r == rj - 1),
                            )
                        osb = opool.tile([P, D_OUT], F32, tag="osb")
                        nc.scalar.activation(
                            out=osb[:], in_=ap[:],
                            func=mybir.ActivationFunctionType.Relu,
                            scale=ndst_sb[y][:, j : j + 1],
                        )
                        nc.sync.dma_start(
                            out_d[y * shard_pad + j * P : y * shard_pad + (j + 1) * P, :],
                            osb[:],
                        )
    nc.compile()
    return nc


def kernel(h, norm, weight, src, dst):
    in_maps, meta, shards = _plan(h, norm, weight, src, dst)
    nc = _build_nc(meta)
    res = run_bass_kernel_spmd(nc, in_maps, core_ids=list(range(meta["n_cores"])))
    out = np.zeros((h.shape[0], D_OUT), dtype=np.float32)
    for c in range(meta["n_cores"]):
        o = res.results[c]["out"]
        for y in range(meta["shards_per_core"]):
            s = c * meta["shards_per_core"] + y
            sd = shards[s]
            rows = o[y * meta["shard_pad"] : y * meta["shard_pad"] + sd["n_local"]]
            out[s * meta["shard_n"] + sd["perm"]] = rows
    return out


# revision 10
# speedup vs baseline: 1.4965x; 1.4965x over previous
"""GCN layer (project -> degree-norm -> gather/scatter-sum -> norm -> relu) on 8 trn2 cores.

Strategy (dst-sharded, per the sharding hint, but with gather-side replication
instead of an all-to-all):
  - 16 dst-shards of 3125 nodes; core c owns shards 2c and 2c+1.
  - Per shard, host renumbers the referenced src nodes compactly (~31.6k distinct,
    +1 zero token) so gather indices fit dma_gather's int16 contract.
  - Device per shard: project h_compact @ W (bf16, PE) -> *norm -> bf16 node table
    in DRAM [NTOK, 128]; SWDGE dma_gather pulls per-edge rows (256B) into SBUF
    in (dst-tile, round) slot order; PE accumulates each dst tile's rounds in
    PSUM via an identity-stationary matmul; ScalarE fuses *norm_dst + ReLU.
  - Edges land in a dense (tile, round, slot) grid: slot = rank of dst within its
    128-node tile (nodes sorted by in-degree so round counts are even), round =
    per-dst edge occurrence. Holes point at token 0 == zero row, so the PSUM sum
    is exact with no scatter collisions anywhere.
All float math happens on device; the host only shards/permutes/casts.
"""

import math

import numpy as np
import ml_dtypes

import concourse.bacc as bacc
import concourse.bass as bass
import concourse.mybir as mybir
import concourse.tile as tile
from concourse.bass_utils import run_bass_kernel_spmd

N_NODES = 50000
D_IN = 256
D_OUT = 96
N_CORES = 8
SHARDS_PER_CORE = 2
P = 128
MAX_GROUPS_PER_BATCH = 64

BF16 = mybir.dt.bfloat16
F32 = mybir.dt.float32
I16 = mybir.dt.int16


def _plan(h, norm, weight, src, dst, n_cores=N_CORES, shards_per_core=SHARDS_PER_CORE):
    """Host-side index/layout preprocessing. Returns per-core input maps plus the
    metadata needed to build the (single, SPMD-shared) bass program and to
    reassemble the full output."""
    n_shards = n_cores * shards_per_core
    n_nodes = h.shape[0]
    shard_n = int(math.ceil(n_nodes / n_shards))
    n_dtiles = int(math.ceil(shard_n / P))
    shard_pad = n_dtiles * P

    src = np.asarray(src).astype(np.int64).ravel()
    dst = np.asarray(dst).astype(np.int64).ravel()
    norm_f = np.asarray(norm, dtype=np.float32).reshape(n_nodes)
    hT = np.ascontiguousarray(np.asarray(h, dtype=np.float32).T)  # [D_IN, n_nodes]

    shard_of = dst // shard_n

    shards = []
    max_ntok = 0
    R_shard = np.zeros((n_shards, n_dtiles), dtype=np.int64)
    for s in range(n_shards):
        lo = s * shard_n
        sel = shard_of == s
        es = src[sel]
        ed = dst[sel] - lo
        n_local = min(shard_n, n_nodes - lo)
        deg = np.bincount(ed, minlength=n_local)
        perm = np.argsort(-deg, kind="stable")  # degree desc
        rank = np.empty(n_local, dtype=np.int64)
        rank[perm] = np.arange(n_local)
        er = rank[ed]
        eorder = np.argsort(er, kind="stable")
        es2 = es[eorder]
        er2 = er[eorder]
        # occurrence index of each edge within its dst's run
        if len(er2):
            starts = np.r_[0, np.flatnonzero(np.diff(er2)) + 1]
            counts = np.diff(np.r_[starts, len(er2)])
            occ = np.arange(len(er2)) - np.repeat(starts, counts)
        else:
            occ = np.zeros(0, dtype=np.int64)
        tj = er2 // P
        np.maximum.at(R_shard[s], tj, occ + 1)
        uniq = np.unique(es2)
        ntok = len(uniq) + 1
        assert ntok <= 32767, f"shard {s}: {ntok} tokens exceeds int16 gather range"
        max_ntok = max(max_ntok, ntok)
        tok = np.searchsorted(uniq, es2) + 1
        shards.append(dict(perm=perm, uniq=uniq, tj=tj, slot=er2 % P, occ=occ,
                           tok=tok, n_local=n_local))

    R_j = np.maximum(R_shard.max(axis=0), 1)  # uniform rounds grid across shards
    goff = np.concatenate([[0], np.cumsum(R_j)])
    G = int(goff[-1])
    NTOK = int(math.ceil(max_ntok / (4 * P)) * 4 * P)  # 4-tile DMA batching
    T_PROJ = NTOK // P

    # batches of whole dst tiles, each <= MAX_GROUPS_PER_BATCH groups
    batches = []  # (g0, [(j, R_j, local_off)...], ngroups)
    cur, cum = [], 0
    for j in range(n_dtiles):
        r = int(R_j[j])
        if cur and cum + r > MAX_GROUPS_PER_BATCH:
            batches.append((int(goff[cur[0]]), [(jj, int(R_j[jj]), int(goff[jj] - goff[cur[0]])) for jj in cur], cum))
            cur, cum = [], 0
        cur.append(j)
        cum += r
    if cur:
        batches.append((int(goff[cur[0]]), [(jj, int(R_j[jj]), int(goff[jj] - goff[cur[0]])) for jj in cur], cum))

    # per-shard device arrays
    for s, sd in enumerate(shards):
        uniq = sd["uniq"]
        ntok = len(uniq) + 1
        hc = np.zeros((D_IN, NTOK), dtype=np.float32)
        hc[:, 1:ntok] = hT[:, uniq]
        # [T_PROJ//4, 128(k), 4(q), 2(j), 128(tok)]: 4 projection tiles per DMA
        ht4 = hc.reshape(2, P, T_PROJ // 4, 4, P).transpose(2, 1, 3, 0, 4)
        ht_tiled = np.ascontiguousarray(ht4).astype(ml_dtypes.bfloat16)

        nsc = np.zeros(NTOK, dtype=np.float32)
        nsc[1:ntok] = norm_f[uniq]
        nsrc = np.ascontiguousarray(nsc.reshape(T_PROJ, P).T)  # [128, T_PROJ]

        nd = np.zeros(shard_pad, dtype=np.float32)
        nd[: sd["n_local"]] = norm_f[s * shard_n + sd["perm"]]
        ndst = np.ascontiguousarray(nd.reshape(n_dtiles, P).T)  # [128, n_dtiles]

        idx_flat = np.zeros(G * P, dtype=np.int64)
        pos = (goff[sd["tj"]] + sd["occ"]) * P + sd["slot"]
        idx_flat[pos] = sd["tok"]
        w16 = idx_flat.reshape(G * 8, 16).T.astype(np.int16)  # [16, G*8]
        gidx = np.ascontiguousarray(np.tile(w16, (8, 1)))  # [128, G*8] replicated

        sd["ht"] = ht_tiled
        sd["nsrc"] = nsrc
        sd["ndst"] = ndst
        sd["gidx"] = gidx

    w2 = np.zeros((D_IN, P), dtype=np.float32)
    w2[:, :D_OUT] = np.asarray(weight, dtype=np.float32)
    # [128(k), 2(j), 128(col)] so partition dim (k within chunk) comes first
    wpad = np.ascontiguousarray(
        w2.reshape(2, P, P).transpose(1, 0, 2)
    ).astype(ml_dtypes.bfloat16)
    ident = np.eye(P, dtype=np.float32).astype(ml_dtypes.bfloat16)

    in_maps = []
    for c in range(n_cores):
        m = {"w": wpad, "ident": ident}
        for y in range(shards_per_core):
            sd = shards[c * shards_per_core + y]
            m[f"ht{y}"] = sd["ht"]
            m[f"nsrc{y}"] = sd["nsrc"]
            m[f"ndst{y}"] = sd["ndst"]
            m[f"gidx{y}"] = sd["gidx"]
        in_maps.append(m)

    meta = dict(
        n_cores=n_cores, shards_per_core=shards_per_core, n_shards=n_shards,
        shard_n=shard_n, n_dtiles=n_dtiles, shard_pad=shard_pad,
        NTOK=NTOK, T_PROJ=T_PROJ, G=G, batches=batches,
    )
    return in_maps, meta, shards


def _build_nc(meta):
    spc = meta["shards_per_core"]
    T_PROJ = meta["T_PROJ"]
    NTOK = meta["NTOK"]
    G = meta["G"]
    n_dtiles = meta["n_dtiles"]
    shard_pad = meta["shard_pad"]
    batches = meta["batches"]
    max_b_groups = max(b[2] for b in batches)

    nc = bacc.Bacc("TRN2", target_bir_lowering=False, debug=False)

    w_d = nc.dram_tensor("w", [P, 2, P], BF16, kind="ExternalInput")
    id_d = nc.dram_tensor("ident", [P, P], BF16, kind="ExternalInput")
    ht_d, nsrc_d, ndst_d, gidx_d = [], [], [], []
    for y in range(spc):
        ht_d.append(nc.dram_tensor(f"ht{y}", [T_PROJ // 4, P, 4, 2, P], BF16, kind="ExternalInput"))
        nsrc_d.append(nc.dram_tensor(f"nsrc{y}", [P, T_PROJ], F32, kind="ExternalInput"))
        ndst_d.append(nc.dram_tensor(f"ndst{y}", [P, n_dtiles], F32, kind="ExternalInput"))
        gidx_d.append(nc.dram_tensor(f"gidx{y}", [P, G * 8], I16, kind="ExternalInput"))
    out_d = nc.dram_tensor("out", [spc * shard_pad, D_OUT], F32, kind="ExternalOutput")

    with tile.TileContext(nc) as tc:
        with (
            tc.tile_pool(name="const", bufs=1) as cpool,
            tc.tile_pool(name="ht", bufs=4) as htpool,
            tc.tile_pool(name="stage", bufs=4) as stpool,
            tc.tile_pool(name="msgs", bufs=4) as mpool,
            tc.tile_pool(name="outsb", bufs=4) as opool,
            tc.tile_pool(name="ppsum", bufs=3, space="PSUM") as pppool,
            tc.tile_pool(name="apsum", bufs=3, space="PSUM") as appool,
            tc.tile_pool(name="dram", bufs=1, space="DRAM") as dpool,
        ):
            w_sb = cpool.tile([P, 2, P], BF16, tag="w")
            nc.sync.dma_start(w_sb[:], w_d[:])
            id_sb = cpool.tile([P, P], BF16, tag="ident")
            nc.sync.dma_start(id_sb[:], id_d[:])
            nsrc_sb, ndst_sb, gidx_sb = [], [], []
            for y in range(spc):
                t1 = cpool.tile([P, T_PROJ], F32, tag=f"nsrc{y}")
                nc.sync.dma_start(t1[:], nsrc_d[y][:])
                nsrc_sb.append(t1)
                t2 = cpool.tile([P, n_dtiles], F32, tag=f"ndst{y}")
                nc.sync.dma_start(t2[:], ndst_d[y][:])
                ndst_sb.append(t2)
                t3 = cpool.tile([P, G * 8], I16, tag=f"gidx{y}")
                nc.sync.dma_start(t3[:], gidx_d[y][:])
                gidx_sb.append(t3)

            for y in range(spc):
                table = dpool.tile([NTOK, P], BF16, tag=f"table{y}")
                # projection: table[tok] = bf16((h_compact @ W) * norm_src)
                for b in range(T_PROJ // 4):
                    ht_t = htpool.tile([P, 4, 2, P], BF16, tag="ht")
                    nc.sync.dma_start(ht_t[:], ht_d[y][b])
                    stg = stpool.tile([P, 4, P], BF16, tag="stage")
                    for q in range(4):
                        t = b * 4 + q
                        pp = pppool.tile([P, P], F32, tag="pp")
                        for j in range(2):
                            nc.tensor.matmul(
                                out=pp[:], lhsT=ht_t[:, q, j, :], rhs=w_sb[:, j, :],
                                start=(j == 0), stop=(j == 1),
                            )
                        nc.vector.tensor_scalar(
                            out=stg[:, q, :], in0=pp[:],
                            scalar1=nsrc_sb[y][:, t : t + 1],
                            scalar2=None, op0=mybir.AluOpType.mult,
                        )
                    # 4 token tiles per write: rows (4b+q)*128+p <- stg[p, q, :]
                    nc.scalar.dma_start(
                        table[b * 4 * P : (b + 1) * 4 * P, :].rearrange(
                            "(q p) c -> p q c", p=P
                        ),
                        stg[:],
                    )

                # gather + per-dst-tile segment sum + norm + relu + store
                for g0, tl, ng in batches:
                    msgs = mpool.tile([P, max_b_groups, P], BF16, tag="msgs")
                    n_idx = ng * P
                    nc.gpsimd.dma_gather(
                        msgs[:, :ng, :], table[:], gidx_sb[y][:, g0 * 8 : (g0 + ng) * 8],
                        n_idx, n_idx, P, single_packet=False,
                    )
                    for j, rj, loff in tl:
                        ap = appool.tile([P, D_OUT], F32, tag="ap")
                        for r in range(rj):
                            nc.tensor.matmul(
                                out=ap[:], lhsT=id_sb[:], rhs=msgs[:, loff + r, 0:D_OUT],
                                start=(r == 0), stop=(r == rj - 1),
                            )
                        osb = opool.tile([P, D_OUT], F32, tag="osb")
                        nc.scalar.activation(
                            out=osb[:], in_=ap[:],
                            func=mybir.ActivationFunctionType.Relu,
                            scale=ndst_sb[y][:, j : j + 1],
                        )
                        nc.scalar.dma_start(
                            out_d[y * shard_pad + j * P : y * shard_pad + (j + 1) * P, :],
                            osb[:],
                        )
    nc.compile()
    return nc


def kernel(h, norm, weight, src, dst):
    in_maps, meta, shards = _plan(h, norm, weight, src, dst)
    nc = _build_nc(meta)
    res = run_bass_kernel_spmd(nc, in_maps, core_ids=list(range(meta["n_cores"])))
    out = np.zeros((h.shape[0], D_OUT), dtype=np.float32)
    for c in range(meta["n_cores"]):
        o = res.results[c]["out"]
        for y in range(meta["shards_per_core"]):
            s = c * meta["shards_per_core"] + y
            sd = shards[s]
            rows = o[y * meta["shard_pad"] : y * meta["shard_pad"] + sd["n_local"]]
            out[s * meta["shard_n"] + sd["perm"]] = rows
    return out
